# revision 1
# baseline (speedup 1.0000x reference)
"""ByteDecoder Trainium2 kernel — 8-core SPMD.

Sharding: core c -> batch b=c//4, query chunk qc=c%4 (256 tokens each).
Per layer each core computes its own K/V slice, AllGathers K/V inside its
4-core batch group, and runs attention/FFN for its 256 tokens.

Host prep: embedding lookup, weight transposes (+LN-gain folding),
attention masks. Device: token-major residual stream, f32r matmuls,
scores computed transposed (S^T[kv,q]) so softmax sums come from a
ones-matmul and normalization is fused into the PSUM eviction.
"""

import sys

for _p in ("/opt/trn_rl_repo", "/root/.axon_site/_ro/trn_rl_repo"):
    if _p not in sys.path:
        sys.path.insert(0, _p)

import numpy as np

import concourse.bass as bass
import concourse.mybir as mybir
import concourse.tile as tile
from concourse import bacc
from concourse.bass_utils import run_bass_kernel_spmd
from concourse.masks import make_identity

F32 = mybir.dt.float32
F32R = mybir.dt.float32r
BF16 = mybir.dt.bfloat16
DT = BF16            # matmul dtype (weights/activations); PSUM stays fp32
AF = mybir.ActivationFunctionType
MUL = mybir.AluOpType.mult
ADD = mybir.AluOpType.add

B, S, D, H, L, W, P, V = 2, 1024, 1024, 16, 9, 1024, 128, 256
DH = D // H          # 64
T = 256              # tokens per core
KD = D // 128        # 8  feature chunks
NK = S // 128        # 8  kv chunks (full sequence)
FH = 4 * D           # 4096
NHD = FH // 128      # 32 hidden chunks
RANKS = 4
EPS = 1e-5



def _build():
    nc = bacc.Bacc("TRN2", target_bir_lowering=False, debug=False,
                   num_devices=8)

    x0 = nc.declare_dram_parameter("x0", [T, D], F32, isOutput=False)
    patchesT = nc.declare_dram_parameter("patchesT", [D, P], DT, isOutput=False)
    camask = nc.declare_dram_parameter("camask", [P, T], DT, isOutput=False)
    samask = nc.declare_dram_parameter("samask", [NK, 128, T], DT, isOutput=False)
    # concatenated per-layer weight blocks (host pre-transposed, bf16):
    #   wqkvT [L, D, 3D] ; scaT = cqT|ckvT [L, D, 3D] ; woT = swoT|cwoT [L, D, 2D]
    #   w1T [L, D, FH] ; w2T [L, FH, D] ; outT [D, V]
    wqkvT = nc.declare_dram_parameter("wqkvT", [L, D, 3 * D], DT, isOutput=False)
    scaT = nc.declare_dram_parameter("scaT", [L, D, 3 * D], DT, isOutput=False)
    woT = nc.declare_dram_parameter("woT", [L, D, 2 * D], DT, isOutput=False)
    w1T = nc.declare_dram_parameter("w1T", [L, D, FH], DT, isOutput=False)
    w2T = nc.declare_dram_parameter("w2T", [L, FH, D], DT, isOutput=False)
    outT = nc.declare_dram_parameter("outT", [D, V], DT, isOutput=False)
    logits = nc.declare_dram_parameter("logits", [T, V], F32, isOutput=True)

    ag_in = nc.dram_tensor("ag_in", [2, D, T], DT)
    ag_out = nc.dram_tensor("ag_out", [RANKS, 2, D, T], DT)

    with tile.TileContext(nc) as tc:
        with (
            tc.tile_pool(name="const", bufs=1) as constp,
            tc.tile_pool(name="persist", bufs=1) as pers,
            tc.tile_pool(name="big", bufs=1) as bigp,
            tc.tile_pool(name="wpool", bufs=10) as wpool,
            tc.tile_pool(name="work", bufs=3) as workp,
            tc.tile_pool(name="attn", bufs=4) as attnp,
            tc.tile_pool(name="ps", bufs=6, space="PSUM") as psp,
        ):
            def psum(cols=512):
                pt = psp.tile([128, 512], F32, tag="ps", name="ps")
                return pt[:, :cols] if cols != 512 else pt

            def wtile(shape):
                # all weight tiles share one 8KB/partition slot tag
                return wpool.tile(shape, DT, tag="w8k", name="wt")

            ident = constp.tile([128, 128], F32)
            make_identity(nc, ident)
            ones_f = constp.tile([128, 1], F32)
            nc.vector.memset(ones_f, 1.0)
            ones = constp.tile([128, 1], DT)
            nc.vector.tensor_copy(out=ones, in_=ones_f)
            eps_t = constp.tile([128, 1], F32)
            nc.vector.memset(eps_t, EPS)
            camask_sb = constp.tile([128, T], DT)
            nc.sync.dma_start(out=camask_sb, in_=camask[:])
            samask_sb = constp.tile([128, NK, T], DT)
            nc.sync.dma_start(out=samask_sb, in_=samask.rearrange("k p t -> p k t"))
            patches_sb = constp.tile([128, KD, P], DT)
            nc.sync.dma_start(out=patches_sb,
                              in_=patchesT.rearrange("(kd p) t -> p kd t", p=128))

            # persistent activations
            x_sb = pers.tile([128, 2, D], F32)
            nc.sync.dma_start(out=x_sb,
                              in_=x0.rearrange("(a p) d -> p a d", p=128))
            hT = pers.tile([128, KD, T], DT)     # transposed LN output
            QT = pers.tile([128, KD, T], DT)     # transposed Q (SA or CA)
            OT = pers.tile([128, KD, T], DT)     # transposed attn out

            def layer_norm_transpose():
                """LN(x_sb) -> hT (transposed, cast to DT)."""
                for tt in range(2):
                    xv = x_sb[:, tt, :]
                    stats = workp.tile([128, 2, 6], F32, tag="ln_stats")
                    nc.vector.bn_stats(out=stats[:, 0, :], in_=xv[:, 0:512])
                    nc.vector.bn_stats(out=stats[:, 1, :], in_=xv[:, 512:1024])
                    mv = workp.tile([128, 2], F32, tag="ln_mv")
                    nc.vector.bn_aggr(out=mv, in_=stats)
                    rstd = workp.tile([128, 1], F32, tag="ln_rstd")
                    nc.scalar.activation(out=rstd, in_=mv[:, 1:2], func=AF.Sqrt,
                                         bias=eps_t, scale=1.0)
                    nc.vector.reciprocal(out=rstd, in_=rstd)
                    hh = workp.tile([128, D], F32, tag="ln_h", bufs=2)
                    nc.vector.tensor_scalar(
                        out=hh, in0=xv, scalar1=mv[:, 0:1], scalar2=rstd,
                        op0=mybir.AluOpType.subtract, op1=MUL)
                    for kd in range(KD):
                        tp = psum(128)
                        nc.tensor.transpose(tp, hh[:, kd * 128:(kd + 1) * 128], ident)
                        nc.vector.tensor_copy(out=hT[:, kd, tt * 128:(tt + 1) * 128],
                                              in_=tp)

            def load_wk(wT_l, c_lo, c_hi):
                """DMA 8 per-kd tiles [128, c_hi-c_lo]."""
                tiles = []
                for kd in range(KD):
                    wt = wtile([128, c_hi - c_lo])
                    nc.sync.dma_start(
                        out=wt, in_=wT_l[kd * 128:(kd + 1) * 128, c_lo:c_hi])
                    tiles.append(wt)
                return tiles

            def proj_featmajor(evict, wtiles, col0, n_out_chunks, rhs, rhs_free):
                """psum[od] = W[:, col0+od*128 cols] contracted with rhs over kd."""
                for og in range(n_out_chunks // 4):
                    pts = [psum(rhs_free) for _ in range(4)]
                    for kd in range(KD):
                        for oi in range(4):
                            c0 = col0 + og * 512 + oi * 128
                            nc.tensor.matmul(pts[oi], wtiles[kd][:, c0:c0 + 128],
                                             rhs(kd),
                                             start=(kd == 0), stop=(kd == KD - 1))
                    for oi in range(4):
                        evict(og * 4 + oi, pts[oi])

            def proj_tokmajor(evict, wtiles, col0, n_cols, lhsT_src, n_tt):
                """psum[nd][tt] = sum_kd lhsT(kd,tt).T @ W[kd][col0+nd*512 : +512]."""
                nnd = n_cols // 512
                pts = [[psum() for _ in range(n_tt)] for _ in range(nnd)]
                for kd in range(KD):
                    for nd in range(nnd):
                        for tt in range(n_tt):
                            nc.tensor.matmul(
                                pts[nd][tt], lhsT_src(kd, tt),
                                wtiles[kd][:, col0 + nd * 512: col0 + nd * 512 + 512],
                                start=(kd == 0), stop=(kd == KD - 1))
                for nd in range(nnd):
                    for tt in range(n_tt):
                        evict(pts[nd][tt], tt, nd)

            def attention(kT_sb, v_sb, n_kc, mask_of, kv_of, v_of):
                """attention into OT (feat-major, normalized)."""
                for h in range(H):
                    bp = (h % 2) * 64
                    kd = h // 2
                    av = psum(T)[:64, :]
                    sm = psum(T)[:1, :]
                    for kc in range(n_kc):
                        st = psum(T)
                        nc.tensor.matmul(
                            st,
                            kT_sb[bp:bp + 64, kd, kv_of(kc):kv_of(kc) + 128],
                            QT[bp:bp + 64, kd, :],
                            start=True, stop=True)
                        e = attnp.tile([128, T], DT, tag="exps")
                        nc.scalar.activation(out=e, in_=st, func=AF.Exp,
                                             scale=0.125)
                        m = mask_of(kc)
                        if m is not None:
                            nc.vector.tensor_tensor(e, e, m, MUL)
                        nc.tensor.matmul(sm, ones, e,
                                         start=(kc == 0), stop=(kc == n_kc - 1))
                        nc.tensor.matmul(av,
                                         v_sb[:, v_of(kc), h * 64:(h + 1) * 64], e,
                                         start=(kc == 0), stop=(kc == n_kc - 1))
                    r1 = attnp.tile([1, T], F32, tag="recip")
                    nc.vector.reciprocal(out=r1, in_=sm)
                    rb = attnp.tile([64, T], F32, tag="recipb")
                    nc.gpsimd.partition_broadcast(rb, r1)
                    nc.vector.tensor_tensor(OT[bp:bp + 64, kd, :], av, rb, MUL)

            def ev_residual(pt, tt, nd):
                nc.vector.tensor_tensor(
                    x_sb[:, tt, nd * 512:(nd + 1) * 512],
                    x_sb[:, tt, nd * 512:(nd + 1) * 512], pt, ADD)

            hT_lhsT = lambda kd, tt: hT[:, kd, tt * 128:(tt + 1) * 128]
            OT_lhsT = lambda kd, tt: OT[:, kd, tt * 128:(tt + 1) * 128]
            hT_rhs = lambda kd: hT[:, kd, :]

            for l in range(L):
                # ---- self attention ----
                layer_norm_transpose()
                wqkv = load_wk(wqkvT[l], 0, 3 * D)

                # K^T self -> ag_in[0]
                ktv = bigp.tile([128, KD, T], DT, tag="ktself")

                def ev_kt(od, pt, ktv=ktv):
                    nc.vector.tensor_copy(out=ktv[:, od, :], in_=pt)
                proj_featmajor(ev_kt, wqkv, D, KD, hT_rhs, T)
                nc.sync.dma_start(
                    out=ag_in[0].rearrange("(kd p) t -> p kd t", p=128), in_=ktv)

                # V self (token-major [T, D] flat) -> ag_in[1]
                vself = bigp.tile([128, 2, D], DT, tag="vself")

                def ev_v(pt, tt, nd, vself=vself):
                    nc.vector.tensor_copy(out=vself[:, tt, nd * 512:(nd + 1) * 512],
                                          in_=pt)
                proj_tokmajor(ev_v, wqkv, 2 * D, D, hT_lhsT, 2)
                nc.sync.dma_start(
                    out=ag_in[1].rearrange("d t -> (d t)")
                                .rearrange("(a p d) -> p a d", p=128, d=D),
                    in_=vself)
                nc.gpsimd.collective_compute(
                    "AllGather", mybir.AluOpType.bypass,
                    replica_groups=[[0, 1, 2, 3], [4, 5, 6, 7]],
                    ins=[ag_in[:]], outs=[ag_out[:]],
                )

                # Q^T (overlaps AG)
                def ev_q(od, pt):
                    nc.vector.tensor_copy(out=QT[:, od, :], in_=pt)
                proj_featmajor(ev_q, wqkv, 0, KD, hT_rhs, T)

                # CA K/V from patches (overlaps AG); scaT = cqT|ckvT
                sca_kv = load_wk(scaT[l], D, 3 * D)
                kca = bigp.tile([128, KD, P], DT, tag="kca")

                def ev_kca(od, pt, kca=kca):
                    nc.vector.tensor_copy(out=kca[:, od, :], in_=pt)
                proj_featmajor(ev_kca, sca_kv, 0, KD,
                               lambda kd: patches_sb[:, kd, :], P)
                vca = bigp.tile([128, 1, D], DT, tag="vca")

                def ev_vca(pt, tt, nd, vca=vca):
                    nc.vector.tensor_copy(out=vca[:, 0, nd * 512:(nd + 1) * 512],
                                          in_=pt)
                proj_tokmajor(ev_vca, sca_kv, D, D,
                              lambda kd, tt: patches_sb[:, kd, :], 1)

                # gathered K/V -> SBUF
                ktg = bigp.tile([128, KD, S], DT, tag="big32", bufs=3)
                vg = bigp.tile([128, NK, D], DT, tag="big32", bufs=3)
                for r in range(RANKS):
                    nc.sync.dma_start(
                        out=ktg[:, :, r * T:(r + 1) * T],
                        in_=ag_out[r, 0].rearrange("(kd p) t -> p kd t", p=128))
                    nc.sync.dma_start(
                        out=vg[:, r * 2:(r + 1) * 2, :],
                        in_=ag_out[r, 1].rearrange("d t -> (d t)")
                                        .rearrange("(a p d) -> p a d", p=128, d=D))

                attention(ktg, vg, NK,
                          lambda kc: samask_sb[:, kc, :],
                          lambda kc: kc * 128, lambda kc: kc)

                # SA out-proj + residual
                swo = load_wk(woT[l], 0, D)
                proj_tokmajor(ev_residual, swo, 0, D, OT_lhsT, 2)

                # ---- cross attention ----
                layer_norm_transpose()
                sca_q = load_wk(scaT[l], 0, D)
                proj_featmajor(ev_q, sca_q, 0, KD, hT_rhs, T)
                attention(kca, vca, 1,
                          lambda kc: camask_sb[:],
                          lambda kc: 0, lambda kc: 0)
                cwo = load_wk(woT[l], D, 2 * D)
                proj_tokmajor(ev_residual, cwo, 0, D, OT_lhsT, 2)

                # ---- FFN ----
                layer_norm_transpose()
                w1 = load_wk(w1T[l], 0, FH)
                f1T = bigp.tile([128, NHD, T], DT, tag="big32", bufs=3)

                def ev_gelu(od, pt, f1T=f1T):
                    nc.scalar.activation(out=f1T[:, od, :], in_=pt, func=AF.Gelu)
                proj_featmajor(ev_gelu, w1, 0, NHD, hT_rhs, T)

                # w2: contraction over FH; row-grouped tiles [128, 4, D]
                pts = [[psum() for _ in range(2)] for _ in range(2)]
                for g in range(8):
                    w2t = wtile([128, 4, D])
                    nc.sync.dma_start(
                        out=w2t,
                        in_=w2T[l][512 * g:512 * (g + 1)]
                            .rearrange("(kd p) c -> p kd c", p=128))
                    for k4 in range(4):
                        kd = 4 * g + k4
                        for nd in range(2):
                            for tt in range(2):
                                nc.tensor.matmul(
                                    pts[nd][tt],
                                    f1T[:, kd, tt * 128:(tt + 1) * 128],
                                    w2t[:, k4, nd * 512:(nd + 1) * 512],
                                    start=(kd == 0), stop=(kd == NHD - 1))
                for nd in range(2):
                    for tt in range(2):
                        ev_residual(pts[nd][tt], tt, nd)

            # ---- final LN + unembed ----
            layer_norm_transpose()
            ow = constp.tile([128, KD, V], DT)
            nc.sync.dma_start(out=ow,
                              in_=outT.rearrange("(kd p) v -> p kd v", p=128))
            for tt in range(2):
                pt = psum(V)
                for kd in range(KD):
                    nc.tensor.matmul(pt, hT[:, kd, tt * 128:(tt + 1) * 128],
                                     ow[:, kd, :],
                                     start=(kd == 0), stop=(kd == KD - 1))
                lg = workp.tile([128, V], F32, tag="lg")
                nc.vector.tensor_copy(out=lg, in_=pt)
                nc.sync.dma_start(out=logits[tt * 128:(tt + 1) * 128, :], in_=lg)

    nc.finalize()
    return nc


_CACHED = {}


def _get_module():
    if "nc" not in _CACHED:
        _CACHED["nc"] = _build()
    return _CACHED["nc"]


def prep_in_maps(byte_seq, patch_representations, patch_boundaries, byte_emb,
           pos_emb, sa_in_w, sa_in_b, sa_out_w, sa_out_b, ca_in_w, ca_in_b,
           ca_out_w, ca_out_b, ffn_w1, ffn_b1, ffn_w2, ffn_b2, ln_g, ln_b,
           norm_g, norm_b, out_w):
    import ml_dtypes
    bf16 = ml_dtypes.bfloat16
    f = np.asarray
    byte_seq = f(byte_seq)
    pr = f(patch_representations, np.float32)
    pb = f(patch_boundaries)
    for bias in (sa_in_b, sa_out_b, ca_in_b, ca_out_b, ffn_b1, ffn_b2,
                 ln_b, norm_b):
        if np.any(np.asarray(bias) != 0):
            raise NotImplementedError("nonzero biases not supported")

    ln_g = f(ln_g, np.float32)
    cast = lambda a: np.ascontiguousarray(a).astype(bf16)
    wqkvT = cast((f(sa_in_w, np.float32) * ln_g[:, 0][:, None, :])
                 .transpose(0, 2, 1))
    # scaT = cqT | ckvT  (CA in-proj: Q gets ln fold, K/V raw patches)
    ca = f(ca_in_w, np.float32).copy()
    ca[:, :D] *= ln_g[:, 1][:, None, :]
    scaT = cast(ca.transpose(0, 2, 1))
    # woT = swoT | cwoT
    woT = cast(np.concatenate(
        [f(sa_out_w, np.float32).transpose(0, 2, 1),
         f(ca_out_w, np.float32).transpose(0, 2, 1)], axis=2))
    w1T = cast((f(ffn_w1, np.float32) * ln_g[:, 2][:, None, :])
               .transpose(0, 2, 1))
    w2T = cast(f(ffn_w2, np.float32).transpose(0, 2, 1))
    outT = cast((f(out_w, np.float32) * f(norm_g, np.float32)[None, :]).T)

    emb = f(byte_emb, np.float32)[byte_seq] + f(pos_emb, np.float32)[None, :S]

    in_maps = []
    for c in range(8):
        b, qc = c // 4, c % 4
        q0 = qc * T
        qg = q0 + np.arange(T)
        kvg = np.arange(S)
        sam = ((kvg[None, :] <= qg[:, None]) &
               (kvg[None, :] > qg[:, None] - W)).T  # [S, T]
        samask = np.ascontiguousarray(
            sam.reshape(NK, 128, T)).astype(bf16)
        pidx = np.cumsum(pb[b].astype(np.int64))[q0:q0 + T]
        camask = (np.arange(P)[:, None] <= pidx[None, :]).astype(bf16)
        in_maps.append({
            "x0": np.ascontiguousarray(emb[b, q0:q0 + T], np.float32),
            "patchesT": cast(pr[b].T),
            "camask": camask,
            "samask": samask,
            "wqkvT": wqkvT, "scaT": scaT, "woT": woT,
            "w1T": w1T, "w2T": w2T, "outT": outT,
        })
    return in_maps


def kernel(**inputs):
    in_maps = prep_in_maps(**inputs)
    nc = _get_module()
    res = run_bass_kernel_spmd(nc, in_maps, list(range(8)))
    out = np.empty((B, S, V), np.float32)
    for c in range(8):
        b, qc = c // 4, c % 4
        out[b, qc * T:(qc + 1) * T] = res.results[c]["logits"]
    return out

